# revision 7
# baseline (speedup 1.0000x reference)
"""BitLinear (RMSNorm + per-token int8 act fake-quant + ternary weight
fake-quant + matmul) on 8 Trainium2 NeuronCores, token-parallel.

Math:
  activation_quant: q = round(xn * beta), beta = 127/(rms*(amax|xn|+eps));
                    xq = q/beta.
  weight_quant:     w3 = clip(round(w*s_w), -1, 1), s_w = 1/(mean|w|+eps).
  out = xq @ wq.T = (q @ w3) * rowscale,  rowscale = (amax+eps)*(mean|w|+eps)/127.
  q in [-127,127] is exact in bf16; w3 in {-1,0,1} is exact in fp8e4; the
  f32 PSUM accumulation of their products is exact, so the big matmul is
  bit-exact vs the fake-quant reference. round() is exact RNE via the fp32
  magic constant (v + 1.5*2^23) - 1.5*2^23.
  mean|w| is computed on the host during input prep (the host already
  builds transposed/sharded views) and passed in as a [128,1] constant.

Per-core kernel (Tc=2048 tokens, D=2048, F=8192):
  Per token-quarter (4 tiles of 128 tokens): x-row stats (DVE amax, ACT
  Square accum), per-token scalar chain, beta column->row via PE transpose
  (borrowed PSUM slot), ones-matmul broadcast, then qT = round(xT*beta) in
  bf16 (SBUF-resident, [d,t] layout for the matmul LHS).
  Matmul stage: 4 feature passes of 2048 columns; weights ternarized into
  fp8 (double-buffered across passes); per (pass, token-tile): for each of
  16 contraction blocks one weight load + 4 N=512 matmuls sweeping 4 PSUM
  banks (ping-pong, 2 groups in flight); PSUM drains alternate ScalarE /
  VectorE applying rowscale; DMA out.
  Emission interleaves next-quarter stats and next-pass ternarize chunks
  between matmul groups so each engine queue matches the execution
  timeline.  A tile_legalize monkeypatch removes redundant consecutive
  InstLdweights (same stationary operand, uninterrupted by any other PE
  weight change), which the legalizer otherwise emits once per matmul.
"""
import numpy as np
import concourse.bacc as bacc
import concourse.tile as tile
import concourse.mybir as mybir
import concourse.tile as _tilemod
from concourse import masks
from concourse.bass_utils import run_bass_kernel_spmd

Alu = mybir.AluOpType
Act = mybir.ActivationFunctionType
F32 = mybir.dt.float32
BF16 = mybir.dt.bfloat16
FP8 = mybir.dt.float8e4

C = 1.5 * 2.0 ** 23  # fp32 RNE rounding constant
EPS_NORM = 1e-6
EPS_ACT = 1e-5
EPS_W = 1e-5
P = 128
FCH = 512
NB = 4               # PSUM banks per sweep group
NCORES = 8


# ---------------------------------------------------------------------------
# Ldweights dedup: the Tile legalizer splits every non-f32 matmul into
# InstLdweights + InstMatmult even when consecutive matmuls share the same
# stationary operand. Each redundant Ldweights costs ~P/1.2GHz serialized on
# the PE queue. Walking each block's scheduler-ordered list, an InstLdweights
# is dropped when its signature matches the previous kept one and every
# intervening PE instruction is a split (non-self-loading) matmul. f32 /
# transpose matmuls self-load and reset the chain.
# ---------------------------------------------------------------------------
_orig_tile_legalize = None


def _ldw_sig(ldw):
    return (
        str(ldw.ins[0]),
        str(getattr(ldw, "perf_mode", None)),
        str(getattr(ldw, "is_transpose", None)),
        str(getattr(ldw, "tile_position", None)),
        str(getattr(ldw, "tile_size", None)),
    )


def _is_split_matmul(inst):
    if not isinstance(inst, mybir.InstMatmult):
        return False
    if getattr(inst, "is_transpose", None):
        return False
    try:
        dt = inst.ins[0].dtype
    except Exception:
        return False
    return dt not in (mybir.dt.float32, mybir.dt.float32r)


def _ldw_dedup(ordered):
    out = {}
    for bn, insts in ordered.items():
        chain_sig = None
        pending_deps = []
        new_list = []
        for inst in insts:
            if isinstance(inst, mybir.InstLdweights):
                s = _ldw_sig(inst)
                if s == chain_sig:
                    deps = getattr(inst, "dependencies", None)
                    if deps:
                        pending_deps.extend(deps)
                    continue
                chain_sig = s
                new_list.append(inst)
                continue
            if isinstance(inst, mybir.InstMatmult):
                if _is_split_matmul(inst):
                    if pending_deps:
                        deps = list(getattr(inst, "dependencies", None) or [])
                        have = {getattr(d, "name", str(d)) for d in deps}
                        for d in pending_deps:
                            if getattr(d, "name", str(d)) not in have:
                                deps.append(d)
                        inst.dependencies = deps
                        pending_deps = []
                else:
                    chain_sig = None
                new_list.append(inst)
                continue
            if type(inst).__name__ == "InstMatmultMx":
                chain_sig = None
            new_list.append(inst)
        out[bn] = new_list
    return out


def _patched_legalize(ordered, nc):
    return _ldw_dedup(_orig_tile_legalize(ordered, nc))


def _install_ldw_dedup():
    global _orig_tile_legalize
    if _orig_tile_legalize is None:
        _orig_tile_legalize = _tilemod.tile_legalize
        _tilemod.tile_legalize = _patched_legalize


# ---------------------------------------------------------------------------
# Program
# ---------------------------------------------------------------------------
def build_program_v2(Tc, D, F, n_devices=NCORES, g_is_ones=True):
    nT = Tc // P
    nD = D // P
    passW = NB * FCH          # features per pass
    nPass = F // passW
    QT = 4                    # token tiles per quantization quarter
    nQ = nT // QT
    QW = QT * P               # token columns per quarter

    _install_ldw_dedup()

    nc = bacc.Bacc("TRN2", num_devices=n_devices)
    x = nc.dram_tensor("x", [Tc, D], F32, kind="ExternalInput")
    xT = nc.dram_tensor("xT", [D, Tc], F32, kind="ExternalInput")
    wT = nc.dram_tensor("wT", [D, F], F32, kind="ExternalInput")
    wme = nc.dram_tensor("wme", [P, 1], F32, kind="ExternalInput")
    swv = nc.dram_tensor("swv", [P, 1], F32, kind="ExternalInput")
    if not g_is_ones:
        g = nc.dram_tensor("g", [1, D], F32, kind="ExternalInput")
    out = nc.dram_tensor("out", [Tc, F], F32, kind="ExternalOutput")

    osb_bufs = 8 if g_is_ones else 6

    with tile.TileContext(nc) as tc:
        with tc.tile_pool(name="const", bufs=1) as const_pool, \
             tc.tile_pool(name="stats", bufs=1) as stats_pool, \
             tc.tile_pool(name="qres", bufs=1) as qres_pool, \
             tc.tile_pool(name="xi", bufs=2) as xi_pool, \
             tc.tile_pool(name="xtj", bufs=3) as xtj_pool, \
             tc.tile_pool(name="sq", bufs=2) as sq_pool, \
             tc.tile_pool(name="bbc", bufs=2) as bbc_pool, \
             tc.tile_pool(name="wb", bufs=6) as wb_pool, \
             tc.tile_pool(name="wr", bufs=3) as wr_pool, \
             tc.tile_pool(name="w3", bufs=2) as w3_pool, \
             tc.tile_pool(name="osb", bufs=osb_bufs) as osb_pool, \
             tc.tile_pool(name="ps", bufs=2, space="PSUM") as pso:

            # ---- constants / persistent ----
            identf = const_pool.tile([P, P], F32)
            masks.make_identity(nc, identf[:])
            ones_row = const_pool.tile([1, P], F32)
            nc.vector.memset(ones_row[:], 1.0)
            wmeane = const_pool.tile([P, 1], F32)
            nc.sync.dma_start(wmeane[:], wme.ap())
            swinv = const_pool.tile([P, 1], F32)
            nc.sync.dma_start(swinv[:], swv.ap())
            if not g_is_ones:
                gT = const_pool.tile([P, nD], F32)
                nc.sync.dma_start(gT[:],
                                  g.ap().rearrange("a (c p) -> (a p) c", p=P))
                g_bc = const_pool.tile([P, D], F32)
                g_row = const_pool.tile([1, D], F32)
                nc.sync.dma_start(g_row[:], g.ap())
                for st in range(0, D, FCH):
                    pgb = pso.tile([P, FCH], F32, name="pgb", tag="pb0")
                    nc.tensor.matmul(pgb[:], ones_row[:],
                                     g_row[:, st:st + FCH],
                                     start=True, stop=True)
                    nc.scalar.activation(g_bc[:, st:st + FCH], pgb[:],
                                         Act.Copy, bias=0.0, scale=1.0)

            rowscale = stats_pool.tile([P, nT], F32)
            s1 = stats_pool.tile([P, nT], F32)
            s2 = stats_pool.tile([P, nT], F32)
            beta_row = stats_pool.tile([1, Tc], F32)
            qT = []
            for j in range(nD):
                qT.append(qres_pool.tile([P, Tc], BF16, name=f"qT{j}",
                                         tag=f"qT{j}"))
            w3_bufs = {}

            def alloc_w3(p):
                if p not in w3_bufs:
                    w3_bufs[p] = [
                        w3_pool.tile([P, passW], FP8, name=f"w3_{j}_p{p}",
                                     tag=f"w3_{j}")
                        for j in range(nD)]
                return w3_bufs[p]

            def borrow_psum(tag):
                return pso.tile([P, FCH], F32, name=f"bor_{tag}", tag=tag)

            def tern_chunk(p, j, c):
                w3 = alloc_w3(p)
                f0 = p * passW
                wb = wb_pool.tile([P, FCH], F32)
                nc.sync.dma_start(
                    wb[:], wT.ap()[j * P:(j + 1) * P,
                                   f0 + c * FCH:f0 + (c + 1) * FCH])
                wr = wr_pool.tile([P, FCH], F32, tag="wr")
                nc.scalar.activation(wr[:], wb[:], Act.Copy,
                                     bias=C, scale=swinv[:, 0:1])
                nc.scalar.activation(wr[:], wr[:], Act.Copy,
                                     bias=-C, scale=1.0)
                nc.vector.tensor_scalar(
                    w3[j][:, c * FCH:(c + 1) * FCH], wr[:], -1.0, 1.0,
                    op0=Alu.max, op1=Alu.min)

            def tern_chunks(p):
                return [(p, j, c) for j in range(nD)
                        for c in range(passW // FCH)]

            def x_stat(i):
                xi = xi_pool.tile([P, D], F32, tag="xi")
                nc.sync.dma_start(xi[:], x.ap()[i * P:(i + 1) * P, :])
                if g_is_ones:
                    nc.vector.tensor_reduce(s2[:, i:i + 1], xi[:],
                                            axis=mybir.AxisListType.X,
                                            op=Alu.max,
                                            apply_absolute_value=True)
                else:
                    xg = sq_pool.tile([P, max(QW, D)], F32, tag="sq")
                    nc.vector.tensor_tensor(xg[:, :D], xi[:], g_bc[:],
                                            op=Alu.mult)
                    nc.vector.tensor_reduce(s2[:, i:i + 1], xg[:, :D],
                                            axis=mybir.AxisListType.X,
                                            op=Alu.max,
                                            apply_absolute_value=True)
                nc.scalar.activation(xi[:], xi[:], Act.Square, bias=0.0,
                                     scale=1.0, accum_out=s1[:, i:i + 1])

            def quarter_beta_q(h):
                a, b = h * QT, (h + 1) * QT
                s1h, s2h = s1[:, a:b], s2[:, a:b]
                nc.vector.tensor_scalar(s1h, s1h, 1.0 / float(D),
                                        float(EPS_NORM),
                                        op0=Alu.mult, op1=Alu.add)
                nc.scalar.activation(s1h, s1h, Act.Sqrt, bias=0.0, scale=1.0)
                nc.vector.reciprocal(s1h, s1h)          # rinv
                nc.vector.tensor_tensor(s2h, s2h, s1h, op=Alu.mult)
                nc.vector.tensor_scalar(s2h, s2h, float(EPS_ACT), None,
                                        op0=Alu.add)    # ae = amax_n + eps
                nc.vector.tensor_scalar(rowscale[:, a:b], s2h, wmeane[:, 0:1],
                                        1.0 / 127.0, op0=Alu.mult,
                                        op1=Alu.mult)
                nc.vector.reciprocal(s2h, s2h)
                nc.vector.tensor_tensor(s1h, s1h, s2h, op=Alu.mult)
                nc.vector.tensor_scalar(s1h, s1h, 127.0, None, op0=Alu.mult)

                for k, i in enumerate(range(a, b)):
                    prow = borrow_psum(f"pb{k % NB}")
                    nc.tensor.transpose(prow[0:1, 0:P], s1[:, i:i + 1],
                                        identf[:])
                    nc.scalar.activation(beta_row[:, i * P:(i + 1) * P],
                                         prow[0:1, 0:P], Act.Copy, bias=0.0,
                                         scale=1.0)
                pbb = borrow_psum("pb0")
                nc.tensor.matmul(pbb[:, 0:FCH], ones_row[:],
                                 beta_row[:, h * QW:(h + 1) * QW],
                                 start=True, stop=True)
                beta_bc = bbc_pool.tile([P, QW], F32)
                nc.scalar.activation(beta_bc[:], pbb[:, 0:FCH], Act.Copy,
                                     bias=0.0, scale=1.0)

                for j in range(nD):
                    xtj = xtj_pool.tile([P, QW], F32, tag="xtj")
                    nc.sync.dma_start(xtj[:], xT.ap()[j * P:(j + 1) * P,
                                                      h * QW:(h + 1) * QW])
                    sq = sq_pool.tile([P, QW] if g_is_ones else [P, max(QW, D)],
                                      F32, tag="sq")
                    if g_is_ones:
                        nc.vector.tensor_tensor(sq[:, :QW], xtj[:], beta_bc[:],
                                                op=Alu.mult)
                    else:
                        nc.scalar.activation(sq[:, :QW], xtj[:], Act.Copy,
                                             bias=0.0, scale=gT[:, j:j + 1])
                        nc.vector.tensor_tensor(sq[:, :QW], sq[:, :QW],
                                                beta_bc[:], op=Alu.mult)
                    nc.vector.tensor_scalar(qT[j][:, h * QW:(h + 1) * QW],
                                            sq[:, :QW], C, C,
                                            op0=Alu.add, op1=Alu.subtract)

            def m_group(p, ti):
                w3 = w3_bufs[p]
                f0 = p * passW
                pbs = [pso.tile([P, FCH], F32, name=f"pb{b}", tag=f"pb{b}")
                       for b in range(NB)]
                for j in range(nD):
                    for b in range(NB):
                        nc.tensor.matmul(
                            pbs[b][:], qT[j][:, ti * P:(ti + 1) * P],
                            w3[j][:, b * FCH:(b + 1) * FCH],
                            start=(j == 0), stop=(j == nD - 1))
                for b in range(NB):
                    ost = osb_pool.tile([P, FCH], F32)
                    if b % 2:
                        nc.vector.tensor_scalar(ost[:], pbs[b][:],
                                                rowscale[:, ti:ti + 1], None,
                                                op0=Alu.mult)
                    else:
                        nc.scalar.activation(ost[:], pbs[b][:], Act.Copy,
                                             bias=0.0,
                                             scale=rowscale[:, ti:ti + 1])
                    nc.sync.dma_start(
                        out.ap()[ti * P:(ti + 1) * P,
                                 f0 + b * FCH:f0 + (b + 1) * FCH], ost[:])

            # ---- emission ----
            # quarter-0 activations first so their DMAs precede the 16.8MB
            # pass-0 weight stream in the sync queue; ternarize follows.
            for i in range(QT):
                x_stat(i)
            quarter_beta_q(0)
            for cch in tern_chunks(0):
                tern_chunk(*cch)

            # pass 0, interleaved with remaining quarters + pass-1 ternarize
            t_next = tern_chunks(1) if nPass > 1 else []
            for h in range(nQ):
                a = h * QT
                for k, ti in enumerate(range(a, a + QT)):
                    m_group(0, ti)
                    gidx = h * QT + k
                    for cch in t_next[gidx * 4:(gidx + 1) * 4]:
                        tern_chunk(*cch)
                    if h + 1 < nQ:
                        if k == 0:
                            x_stat((h + 1) * QT)
                            x_stat((h + 1) * QT + 1)
                        elif k == 1:
                            x_stat((h + 1) * QT + 2)
                        elif k == 2:
                            x_stat((h + 1) * QT + 3)
                            quarter_beta_q(h + 1)

            for p in range(1, nPass):
                t_next = tern_chunks(p + 1) if p + 1 < nPass else []
                for ti in range(nT):
                    m_group(p, ti)
                    for cch in t_next[ti * 4:(ti + 1) * 4]:
                        tern_chunk(*cch)

    nc.compile()
    return nc


_prog_cache = {}


def _get_program(Tc, D, F, g_is_ones=True):
    key = (Tc, D, F, g_is_ones)
    if key not in _prog_cache:
        _prog_cache[key] = build_program_v2(Tc, D, F, g_is_ones=g_is_ones)
    return _prog_cache[key]


def make_in_maps(x, norm_weight, weight):
    B, S, D = x.shape
    F = weight.shape[0]
    T = B * S
    Tc = T // NCORES
    g_is_ones = bool(np.all(norm_weight == 1.0))
    xf = np.ascontiguousarray(x.reshape(T, D), dtype=np.float32)
    wTv = np.ascontiguousarray(weight.T).astype(np.float32, copy=False)
    wm = np.float32(np.abs(weight).mean(dtype=np.float64)) + np.float32(EPS_W)
    wme_v = np.full((P, 1), wm, np.float32)
    swv_v = np.full((P, 1), np.float32(1.0) / wm, np.float32)
    gv = np.ascontiguousarray(norm_weight.reshape(1, D), dtype=np.float32)
    in_maps = []
    for c in range(NCORES):
        xs = xf[c * Tc:(c + 1) * Tc]
        m = {
            "x": xs,
            "xT": np.ascontiguousarray(xs.T),
            "wT": wTv,
            "wme": wme_v,
            "swv": swv_v,
        }
        if not g_is_ones:
            m["g"] = gv
        in_maps.append(m)
    return in_maps, (B, S, T, Tc, D, F, g_is_ones)


def kernel(x, norm_weight, weight):
    x = np.asarray(x)
    norm_weight = np.asarray(norm_weight)
    weight = np.asarray(weight)
    in_maps, (B, S, T, Tc, D, F, g_is_ones) = make_in_maps(
        x, norm_weight, weight)
    nc = _get_program(Tc, D, F, g_is_ones=g_is_ones)
    res = run_bass_kernel_spmd(nc, in_maps, core_ids=list(range(NCORES)))
    outp = np.concatenate([res.results[c]["out"] for c in range(NCORES)],
                          axis=0)
    return np.ascontiguousarray(
        outp.reshape(B, S, F).astype(np.float32, copy=False))
